# revision 15
# baseline (speedup 1.0000x reference)
"""Gated max/avg 2x2 pooling kernel for Trainium2 (8 NeuronCores, SPMD).

Reference computation (per 2x2 window over [B, H, W, C], stride 2):
    x1 = max(window), x2 = mean(window)
    xs = sum_ij mask[i, j] * window[i, j]   (per channel)
    z  = sigmoid(xs)
    out = z * x1 + (1 - z) * x2

Sharding: pure data-parallel over batch (16 batches -> 2 per core).

Engine split (per 1024-col tile, partition dim = 128 output rows):
  PE    xs path: 4 diagonal-stationary matmuls (u_k * I) accumulating
        sum_k u_k*T_k in PSUM (T_k = the 4 window terms Ee,Eo,Oe,Oo);
        on alternating tiles also the mean path with a 0.25*I stationary.
  ACT   sigmoid straight out of PSUM (free scale f restores mask norm);
        PSUM->SBUF fp16 copy of the mean.
  DVE   max pool (2 tensor_max) + gating (mul + add), all fp16 @2x.
  GPSIMD  d = x1 - s (one tensor_sub).
Inputs are staged to the device as fp16 (host converts), output returns
fp16 and is upcast on the host: HBM traffic drops 2x vs f32.
"""

import numpy as np

import concourse.bacc as bacc
import concourse.mybir as mybir
import concourse.tile as tile
from concourse.bass_utils import run_bass_kernel_spmd

F32 = mybir.dt.float32
F16 = mybir.dt.float16

B, H, W, C = 16, 256, 256, 64
N_CORES = 8
BPC = B // N_CORES          # batches per core
HO = H // 2                 # 128 output rows = SBUF partitions
NQ = 8                      # w-slices (tiles) per row
WQ = 16                     # output w per tile
N = WQ * C                  # 1024 free elems per partition per output tile
FD = 4 * N                  # input tile free dim (r2 * w16 * e2 * c64)
MMCH = 512                  # matmul moving-free chunk (PSUM bank)

# Tiles whose mean path runs on PE (rest on DVE): i % PERIOD != PERIOD-1.
SUM_PE_PERIOD = 4           # 3 of 4 tiles on PE

LAST_EXEC_NS = None
LAST_RESULTS = None

_PROGRAM_CACHE = {}


def _build_program(bpc, nq, wq, ch):
    from contextlib import ExitStack

    n = wq * ch
    fd = 4 * n
    nch = n // MMCH             # psum chunks per tile

    nc = bacc.Bacc(
        "TRN2",
        target_bir_lowering=False,
        debug=False,
        enable_asserts=True,
        num_devices=N_CORES,
    )

    x = nc.dram_tensor("x", [bpc, HO, nq, fd], F16, kind="ExternalInput")
    wmat = nc.dram_tensor("wmat", [128, 5 * 128], F16, kind="ExternalInput")
    scal = nc.dram_tensor("scal", [128, 8], F32, kind="ExternalInput")
    out = nc.dram_tensor("out", [bpc, HO, nq, n], F16, kind="ExternalOutput")
    x_ap = x.ap()
    out_ap = out.ap()

    with tile.TileContext(nc) as tc, ExitStack() as ctx:
        pio = ctx.enter_context(tc.tile_pool(name="io", bufs=3))
        pbig = ctx.enter_context(tc.tile_pool(name="big", bufs=2))
        psm = ctx.enter_context(tc.tile_pool(name="small", bufs=2))
        pout = ctx.enter_context(tc.tile_pool(name="outp", bufs=2))
        pconst = ctx.enter_context(tc.tile_pool(name="const", bufs=1))
        ppsum = ctx.enter_context(tc.tile_pool(name="acc", bufs=2, space="PSUM"))

        Wt = pconst.tile([128, 5 * 128], F16)
        nc.sync.dma_start(Wt[:], wmat.ap()[:])
        Sc = pconst.tile([128, 8], F32)
        nc.sync.dma_start(Sc[:], scal.ap()[:])
        f_ap = Sc[:, 0:1]
        zero_ap = Sc[:, 1:2]
        q25_ap = Sc[:, 2:3]
        Wd = [Wt[:, k * 128 : (k + 1) * 128] for k in range(5)]

        def emit_load(b, q, nqs):
            """Load a superblock of nqs q-slices as one DMA (bigger
            descriptors -> better HBM efficiency)."""
            SB = pio.tile([128, nqs * fd], F16, tag=f"SB{nqs}")
            nc.sync.dma_start(
                SB[:].rearrange("p (q f) -> p q f", q=nqs),
                x_ap[b, :, q : q + nqs, :],
            )
            return [dict(b=b, q=q + j, EO=SB[:, j * fd : (j + 1) * fd]) for j in range(nqs)]

        def emit_compute(h, sum_on_pe):
            b, q, EO = h["b"], h["q"], h["EO"]
            EOv = EO.rearrange("p (r w e c) -> p r w e c", r=2, e=2, c=ch)
            wpc = MMCH // ch    # moving w's per chunk

            def term(k, c0=0, nw=wq):
                r, e = divmod(k, 2)
                return EOv[:, r, c0 : c0 + nw, e, :]

            # mean path first (frees ACT to copy s while xs still runs):
            # s = (Ee+Eo+Oe+Oo)/4 via PE with 0.25*I stationary
            s025 = psm.tile([128, n], F16, tag="s025")
            if sum_on_pe:
                ps = ppsum.tile([128, n], F32, tag="ps")
                for cH in range(nch):
                    for k in range(4):
                        nc.tensor.matmul(
                            ps[:, cH * MMCH : (cH + 1) * MMCH],
                            Wd[4],
                            term(k, cH * wpc, wpc),
                            start=(k == 0),
                            stop=(k == 3),
                        )
                nc.scalar.copy(s025[:], ps[:])
            else:
                # e-major intermediate: pair-add reads contiguous halves
                S1 = pbig.tile([128, 2 * n], F16, tag="S1")
                nc.vector.tensor_add(
                    S1[:].rearrange("p (e w c) -> p w e c", e=2, c=ch),
                    EOv[:, 0],
                    EOv[:, 1],
                )
                s4 = psm.tile([128, n], F16, tag="s4")
                nc.vector.tensor_add(s4[:], S1[:, 0:n], S1[:, n : 2 * n])
                nc.scalar.mul(s025[:], s4[:], q25_ap)

            # xs path on PE: psum[:, chunk] = sum_k u_k * T_k
            pxs = ppsum.tile([128, n], F32, tag="pxs")
            for cH in range(nch):
                for k in range(4):
                    nc.tensor.matmul(
                        pxs[:, cH * MMCH : (cH + 1) * MMCH],
                        Wd[k],
                        term(k, cH * wpc, wpc),
                        start=(k == 0),
                        stop=(k == 3),
                    )
            z = psm.tile([128, n], F16, tag="z")
            nc.scalar.activation(
                z[:],
                pxs[:],
                mybir.ActivationFunctionType.Sigmoid,
                bias=zero_ap,
                scale=f_ap,
            )

            # max path on DVE, e-major intermediate
            M1 = pbig.tile([128, 2 * n], F16, tag="M1")
            nc.vector.tensor_max(
                M1[:].rearrange("p (e w c) -> p w e c", e=2, c=ch),
                EOv[:, 0],
                EOv[:, 1],
            )
            x1 = psm.tile([128, n], F16, tag="x1")
            nc.vector.tensor_max(x1[:], M1[:, 0:n], M1[:, n : 2 * n])

            # gating: out = s + z*(x1 - s); the sub rides on GPSIMD
            d = psm.tile([128, n], F16, tag="d")
            nc.gpsimd.tensor_sub(d[:], x1[:], s025[:])
            g = psm.tile([128, n], F16, tag="g")
            nc.vector.tensor_mul(g[:], z[:], d[:])
            o = pout.tile([128, n], F16, tag="o")
            nc.vector.tensor_add(o[:], s025[:], g[:])
            nc.sync.dma_start(out_ap[b, :, q, :], o[:])

        sbs = [(b, q) for b in range(bpc) for q in range(0, nq, 2)]
        ntiles = bpc * nq
        # first superblock split into single-tile loads (shorter startup)
        pending = emit_load(sbs[0][0], sbs[0][1], 1)
        pending += emit_load(sbs[0][0], sbs[0][1] + 1, 1)
        next_sb = 1
        if next_sb < len(sbs):
            pending += emit_load(*sbs[next_sb], 2)
            next_sb += 1
        for i in range(ntiles):
            if next_sb < len(sbs) and len(pending) <= 3:
                pending += emit_load(*sbs[next_sb], 2)
                next_sb += 1
            emit_compute(
                pending.pop(0),
                sum_on_pe=(i % SUM_PE_PERIOD != SUM_PE_PERIOD - 1),
            )

    nc.compile()
    return nc


def _get_program(key):
    if key not in _PROGRAM_CACHE:
        _PROGRAM_CACHE[key] = _build_program(*key)
    return _PROGRAM_CACHE[key]


def _mask_consts(mask):
    """wmat [128, 5*128] f16 (diagonal stationaries u0..u3, 0.25*I) and
    scal [128, 8] f32 (sigmoid scale f, 0, 0.25)."""
    m = np.asarray(mask, np.float64).reshape(-1)  # m00 m01 m10 m11 = Ee Eo Oe Oo
    f = float(m[np.argmax(np.abs(m))])
    if f == 0.0:
        f = 1.0
    u = m / f
    wmat = np.zeros((128, 5 * 128), np.float16)
    idx = np.arange(128)
    for k in range(4):
        wmat[idx, k * 128 + idx] = np.float16(u[k])
    wmat[idx, 4 * 128 + idx] = np.float16(0.25)
    scal = np.zeros((128, 8), np.float32)
    scal[:, 0] = f
    scal[:, 2] = 0.25
    return wmat, scal


def kernel(x, mask):
    import os

    global LAST_EXEC_NS, LAST_RESULTS

    x = np.asarray(x)
    mask = np.asarray(mask)
    assert x.shape == (B, H, W, C), x.shape
    in_dtype = x.dtype

    wmat, scal = _mask_consts(mask)
    nc = _get_program((BPC, NQ, WQ, C))

    # stage as [b, h, q, r, w, e, c] -> fp16
    xs = np.asarray(x, np.float32).reshape(B, HO, 2, NQ, WQ, 2, C)
    xt = xs.transpose(0, 1, 3, 2, 4, 5, 6)
    xv = np.ascontiguousarray(xt).astype(np.float16).reshape(B, HO, NQ, FD)

    in_maps = [
        {"x": xv[i * BPC : (i + 1) * BPC], "wmat": wmat, "scal": scal}
        for i in range(N_CORES)
    ]

    trace = os.environ.get("KERNEL_TRACE", "0") == "1"
    res = run_bass_kernel_spmd(
        nc, in_maps, core_ids=list(range(N_CORES)), trace=trace
    )
    LAST_EXEC_NS = res.exec_time_ns
    LAST_RESULTS = res

    parts = [
        r["out"].reshape(BPC, HO, NQ * WQ, C).astype(np.float32)
        for r in res.results
    ]
    full = np.concatenate(parts, axis=0)
    return full.astype(in_dtype, copy=False)


def _numpy_reference(x, mask):
    xr = x.reshape(x.shape[0], x.shape[1] // 2, 2, x.shape[2] // 2, 2, x.shape[3])
    x1 = xr.max(axis=(2, 4))
    x2 = xr.mean(axis=(2, 4))
    xs = np.einsum("bhiwjc,ij->bhwc", xr, mask)
    z = 1.0 / (1.0 + np.exp(-xs))
    return z * x1 + (1.0 - z) * x2


if __name__ == "__main__":
    # Small-scale CoreSim self-test (no hardware needed).
    from concourse.bass_interp import CoreSim

    rng = np.random.default_rng(0)
    bpc_s, nq_s = 1, 2
    w_s = nq_s * WQ * 2
    xs_np = rng.standard_normal((bpc_s, H, w_s, C)).astype(np.float32)
    mask_np = (rng.standard_normal((2, 2)) * 0.5).astype(np.float32)

    wmat_s, scal_s = _mask_consts(mask_np)
    nc = _build_program(bpc_s, nq_s, WQ, C)
    sim = CoreSim(nc, trace=False)
    xr = xs_np.reshape(bpc_s, HO, 2, nq_s, WQ, 2, C).transpose(0, 1, 3, 2, 4, 5, 6)
    sim.tensor("x")[:] = (
        np.ascontiguousarray(xr).astype(np.float16).reshape(bpc_s, HO, nq_s, FD)
    )
    sim.tensor("wmat")[:] = wmat_s
    sim.tensor("scal")[:] = scal_s
    sim.simulate()
    got = sim.tensor("out").reshape(bpc_s, HO, nq_s * WQ, C).astype(np.float64)
    want = _numpy_reference(xs_np.astype(np.float64), mask_np.astype(np.float64))
    err = np.abs(got - want)
    rel = err.max() / np.abs(want).max()
    print("CoreSim selftest: max abs err", err.max(), "rel", rel)
    assert rel < 5e-3, rel
    print("PASS")


# revision 17
# speedup vs baseline: 1.0050x; 1.0050x over previous
"""Gated max/avg 2x2 pooling kernel for Trainium2 (8 NeuronCores, SPMD).

Reference computation (per 2x2 window over [B, H, W, C], stride 2):
    x1 = max(window), x2 = mean(window)
    xs = sum_ij mask[i, j] * window[i, j]   (per channel)
    z  = sigmoid(xs)
    out = z * x1 + (1 - z) * x2

Sharding: pure data-parallel over batch (16 batches -> 2 per core).

Engine split (per 1024-col tile, partition dim = 128 output rows):
  PE    xs path: 4 diagonal-stationary matmuls (u_k * I) accumulating
        sum_k u_k*T_k in PSUM (T_k = the 4 window terms Ee,Eo,Oe,Oo);
        on alternating tiles also the mean path with a 0.25*I stationary.
  ACT   sigmoid straight out of PSUM (free scale f restores mask norm);
        PSUM->SBUF fp16 copy of the mean.
  DVE   max pool (2 tensor_max) + gating (mul + add), all fp16 @2x.
  GPSIMD  d = x1 - s (one tensor_sub).
Inputs are staged to the device as fp16 (host converts), output returns
fp16 and is upcast on the host: HBM traffic drops 2x vs f32.
"""

import numpy as np

import concourse.bacc as bacc
import concourse.mybir as mybir
import concourse.tile as tile
from concourse.bass_utils import run_bass_kernel_spmd

F32 = mybir.dt.float32
F16 = mybir.dt.float16

B, H, W, C = 16, 256, 256, 64
N_CORES = 8
BPC = B // N_CORES          # batches per core
HO = H // 2                 # 128 output rows = SBUF partitions
NQ = 8                      # w-slices (tiles) per row
WQ = 16                     # output w per tile
N = WQ * C                  # 1024 free elems per partition per output tile
FD = 4 * N                  # input tile free dim (r2 * w16 * e2 * c64)
MMCH = 512                  # matmul moving-free chunk (PSUM bank)

# Tiles whose mean path runs on PE (rest on DVE): i % PERIOD != PERIOD-1.
SUM_PE_PERIOD = 1000        # all tiles on PE

LAST_EXEC_NS = None
LAST_RESULTS = None

_PROGRAM_CACHE = {}


def _build_program(bpc, nq, wq, ch):
    from contextlib import ExitStack

    n = wq * ch
    fd = 4 * n
    nch = n // MMCH             # psum chunks per tile

    nc = bacc.Bacc(
        "TRN2",
        target_bir_lowering=False,
        debug=False,
        enable_asserts=True,
        num_devices=N_CORES,
    )

    x = nc.dram_tensor("x", [bpc, HO, nq, fd], F16, kind="ExternalInput")
    wmat = nc.dram_tensor("wmat", [128, 5 * 128], F16, kind="ExternalInput")
    scal = nc.dram_tensor("scal", [128, 8], F32, kind="ExternalInput")
    out = nc.dram_tensor("out", [bpc, HO, nq, n], F16, kind="ExternalOutput")
    x_ap = x.ap()
    out_ap = out.ap()

    with tile.TileContext(nc) as tc, ExitStack() as ctx:
        pio = ctx.enter_context(tc.tile_pool(name="io", bufs=3))
        pbig = ctx.enter_context(tc.tile_pool(name="big", bufs=2))
        psm = ctx.enter_context(tc.tile_pool(name="small", bufs=2))
        pout = ctx.enter_context(tc.tile_pool(name="outp", bufs=2))
        pconst = ctx.enter_context(tc.tile_pool(name="const", bufs=1))
        ppsum = ctx.enter_context(tc.tile_pool(name="acc", bufs=2, space="PSUM"))

        Wt = pconst.tile([128, 5 * 128], F16)
        nc.sync.dma_start(Wt[:], wmat.ap()[:])
        Sc = pconst.tile([128, 8], F32)
        nc.sync.dma_start(Sc[:], scal.ap()[:])
        f_ap = Sc[:, 0:1]
        zero_ap = Sc[:, 1:2]
        q25_ap = Sc[:, 2:3]
        Wd = [Wt[:, k * 128 : (k + 1) * 128] for k in range(5)]

        load_engines = [nc.sync, nc.scalar]
        load_rr = [0]

        def emit_load(b, q, nqs):
            """Load a superblock of nqs q-slices as one DMA (bigger
            descriptors -> better HBM efficiency).  Alternate the issuing
            engine so the transfers spread over two HW DGE queues."""
            SB = pio.tile([128, nqs * fd], F16, tag=f"SB{nqs}")
            eng = load_engines[load_rr[0] % len(load_engines)]
            load_rr[0] += 1
            eng.dma_start(
                SB[:].rearrange("p (q f) -> p q f", q=nqs),
                x_ap[b, :, q : q + nqs, :],
            )
            return [dict(b=b, q=q + j, EO=SB[:, j * fd : (j + 1) * fd]) for j in range(nqs)]

        def emit_compute(h, sum_on_pe):
            b, q, EO = h["b"], h["q"], h["EO"]
            EOv = EO.rearrange("p (r w e c) -> p r w e c", r=2, e=2, c=ch)
            wpc = MMCH // ch    # moving w's per chunk

            def term(k, c0=0, nw=wq):
                r, e = divmod(k, 2)
                return EOv[:, r, c0 : c0 + nw, e, :]

            # mean path first (frees ACT to copy s while xs still runs):
            # s = (Ee+Eo+Oe+Oo)/4 via PE with 0.25*I stationary
            s025 = psm.tile([128, n], F16, tag="s025")
            if sum_on_pe:
                ps = ppsum.tile([128, n], F32, tag="ps")
                for cH in range(nch):
                    for k in range(4):
                        nc.tensor.matmul(
                            ps[:, cH * MMCH : (cH + 1) * MMCH],
                            Wd[4],
                            term(k, cH * wpc, wpc),
                            start=(k == 0),
                            stop=(k == 3),
                        )
                nc.scalar.copy(s025[:], ps[:])
            else:
                # e-major intermediate: pair-add reads contiguous halves
                S1 = pbig.tile([128, 2 * n], F16, tag="S1")
                nc.vector.tensor_add(
                    S1[:].rearrange("p (e w c) -> p w e c", e=2, c=ch),
                    EOv[:, 0],
                    EOv[:, 1],
                )
                s4 = psm.tile([128, n], F16, tag="s4")
                nc.vector.tensor_add(s4[:], S1[:, 0:n], S1[:, n : 2 * n])
                nc.scalar.mul(s025[:], s4[:], q25_ap)

            # xs path on PE: psum[:, chunk] = sum_k u_k * T_k
            pxs = ppsum.tile([128, n], F32, tag="pxs")
            for cH in range(nch):
                for k in range(4):
                    nc.tensor.matmul(
                        pxs[:, cH * MMCH : (cH + 1) * MMCH],
                        Wd[k],
                        term(k, cH * wpc, wpc),
                        start=(k == 0),
                        stop=(k == 3),
                    )
            z = psm.tile([128, n], F16, tag="z")
            nc.scalar.activation(
                z[:],
                pxs[:],
                mybir.ActivationFunctionType.Sigmoid,
                bias=zero_ap,
                scale=f_ap,
            )

            # max path on DVE, e-major intermediate
            M1 = pbig.tile([128, 2 * n], F16, tag="M1")
            nc.vector.tensor_max(
                M1[:].rearrange("p (e w c) -> p w e c", e=2, c=ch),
                EOv[:, 0],
                EOv[:, 1],
            )
            x1 = psm.tile([128, n], F16, tag="x1")
            nc.vector.tensor_max(x1[:], M1[:, 0:n], M1[:, n : 2 * n])

            # gating: out = s + z*(x1 - s); the sub rides on GPSIMD
            d = psm.tile([128, n], F16, tag="d")
            nc.gpsimd.tensor_sub(d[:], x1[:], s025[:])
            g = psm.tile([128, n], F16, tag="g")
            nc.vector.tensor_mul(g[:], z[:], d[:])
            o = pout.tile([128, n], F16, tag="o")
            nc.vector.tensor_add(o[:], s025[:], g[:])
            nc.sync.dma_start(out_ap[b, :, q, :], o[:])

        sbs = [(b, q) for b in range(bpc) for q in range(0, nq, 2)]
        ntiles = bpc * nq
        # first superblock split into single-tile loads (shorter startup)
        pending = emit_load(sbs[0][0], sbs[0][1], 1)
        pending += emit_load(sbs[0][0], sbs[0][1] + 1, 1)
        next_sb = 1
        if next_sb < len(sbs):
            pending += emit_load(*sbs[next_sb], 2)
            next_sb += 1
        for i in range(ntiles):
            if next_sb < len(sbs) and len(pending) <= 3:
                pending += emit_load(*sbs[next_sb], 2)
                next_sb += 1
            emit_compute(
                pending.pop(0),
                sum_on_pe=(i % SUM_PE_PERIOD != SUM_PE_PERIOD - 1),
            )

    nc.compile()
    return nc


def _get_program(key):
    if key not in _PROGRAM_CACHE:
        _PROGRAM_CACHE[key] = _build_program(*key)
    return _PROGRAM_CACHE[key]


def _mask_consts(mask):
    """wmat [128, 5*128] f16 (diagonal stationaries u0..u3, 0.25*I) and
    scal [128, 8] f32 (sigmoid scale f, 0, 0.25)."""
    m = np.asarray(mask, np.float64).reshape(-1)  # m00 m01 m10 m11 = Ee Eo Oe Oo
    f = float(m[np.argmax(np.abs(m))])
    if f == 0.0:
        f = 1.0
    u = m / f
    wmat = np.zeros((128, 5 * 128), np.float16)
    idx = np.arange(128)
    for k in range(4):
        wmat[idx, k * 128 + idx] = np.float16(u[k])
    wmat[idx, 4 * 128 + idx] = np.float16(0.25)
    scal = np.zeros((128, 8), np.float32)
    scal[:, 0] = f
    scal[:, 2] = 0.25
    return wmat, scal


def kernel(x, mask):
    import os

    global LAST_EXEC_NS, LAST_RESULTS

    x = np.asarray(x)
    mask = np.asarray(mask)
    assert x.shape == (B, H, W, C), x.shape
    in_dtype = x.dtype

    wmat, scal = _mask_consts(mask)
    nc = _get_program((BPC, NQ, WQ, C))

    # stage as [b, h, q, r, w, e, c] -> fp16
    xs = np.asarray(x, np.float32).reshape(B, HO, 2, NQ, WQ, 2, C)
    xt = xs.transpose(0, 1, 3, 2, 4, 5, 6)
    xv = np.ascontiguousarray(xt).astype(np.float16).reshape(B, HO, NQ, FD)

    in_maps = [
        {"x": xv[i * BPC : (i + 1) * BPC], "wmat": wmat, "scal": scal}
        for i in range(N_CORES)
    ]

    trace = os.environ.get("KERNEL_TRACE", "0") == "1"
    res = run_bass_kernel_spmd(
        nc, in_maps, core_ids=list(range(N_CORES)), trace=trace
    )
    LAST_EXEC_NS = res.exec_time_ns
    LAST_RESULTS = res

    parts = [
        r["out"].reshape(BPC, HO, NQ * WQ, C).astype(np.float32)
        for r in res.results
    ]
    full = np.concatenate(parts, axis=0)
    return full.astype(in_dtype, copy=False)


def _numpy_reference(x, mask):
    xr = x.reshape(x.shape[0], x.shape[1] // 2, 2, x.shape[2] // 2, 2, x.shape[3])
    x1 = xr.max(axis=(2, 4))
    x2 = xr.mean(axis=(2, 4))
    xs = np.einsum("bhiwjc,ij->bhwc", xr, mask)
    z = 1.0 / (1.0 + np.exp(-xs))
    return z * x1 + (1.0 - z) * x2


if __name__ == "__main__":
    # Small-scale CoreSim self-test (no hardware needed).
    from concourse.bass_interp import CoreSim

    rng = np.random.default_rng(0)
    bpc_s, nq_s = 1, 2
    w_s = nq_s * WQ * 2
    xs_np = rng.standard_normal((bpc_s, H, w_s, C)).astype(np.float32)
    mask_np = (rng.standard_normal((2, 2)) * 0.5).astype(np.float32)

    wmat_s, scal_s = _mask_consts(mask_np)
    nc = _build_program(bpc_s, nq_s, WQ, C)
    sim = CoreSim(nc, trace=False)
    xr = xs_np.reshape(bpc_s, HO, 2, nq_s, WQ, 2, C).transpose(0, 1, 3, 2, 4, 5, 6)
    sim.tensor("x")[:] = (
        np.ascontiguousarray(xr).astype(np.float16).reshape(bpc_s, HO, nq_s, FD)
    )
    sim.tensor("wmat")[:] = wmat_s
    sim.tensor("scal")[:] = scal_s
    sim.simulate()
    got = sim.tensor("out").reshape(bpc_s, HO, nq_s * WQ, C).astype(np.float64)
    want = _numpy_reference(xs_np.astype(np.float64), mask_np.astype(np.float64))
    err = np.abs(got - want)
    rel = err.max() / np.abs(want).max()
    print("CoreSim selftest: max abs err", err.max(), "rel", rel)
    assert rel < 5e-3, rel
    print("PASS")
